# revision 3
# baseline (speedup 1.0000x reference)
"""Trainium2 Bass kernel for style-modulated 3D conv (DMSRStyleConv).

Math (per sample b):
  s[ci]   = style[b] @ style_w.T + style_b                    # [Cin]
  w_mod   = weight * s[None, :, None,None,None]               # [Cout,Cin,3,3,3]
  var[co] = sum_{ci,taps} w_mod^2 = sum_ci s[ci]^2 * q[ci,co]
  y[b]    = conv3d_valid(x[b], w_mod) * rsqrt(var+eps)[co] + bias[co]

Strategy: data-parallel over batch across 8 cores (1 sample each).

PE formulation: all matmuls are full-array K=128 x M=128.  The
depth (d) axis is paired in the contraction dim and the output depth
(od) axis is paired in the PSUM partition dim:

  rhs "D" tile = [x[2e+1] ; x[2e+2]]  (128 partitions = 2 slices x 64 cin)
  rhs "R" tile = [x[2e]   ; x[2e+3]]
  psum tile    = [y[2e]   ; y[2e+1]]  (128 partitions = 2 od x 64 cout)

Per od-pair e and per (kh,kw) there are two passes:
  dense (rhs=D): lhsT = [[W1  W0],[W2  W1]]   (all 4 blocks useful)
  rem   (rhs=R): lhsT = [[W0   0],[ 0  W2]]
giving 18 passes x 2116 spatial columns per od-pair = 876k psum columns
total (vs 2.63M for a 64x64-quadrant formulation).

Weights are static across the sample's conv: 18 base lhsT tiles are
precomputed on the host, modulated once per kernel by s[ci] (one DVE op)
into fp16.  x slices are cast fp32->fp16 during the DMA itself (SWDGE).
Demod scale and bias are folded into the PSUM->SBUF eviction.
"""

import numpy as np

import concourse.bass as bass
import concourse.tile as tile
from concourse import bacc, mybir
from concourse.bass_utils import run_bass_kernel_spmd

F32 = mybir.dt.float32
F16 = mybir.dt.float16
EPS = 1e-8
N_CORES = 8
CIN = 64
COUT = 64
KK = 3
NTAP = 18          # 9 dense + 9 remainder lhsT tiles


def conv_body(ctx, tc, y_ap, x_ap, st4_ap, swt_ap, stb2_ap, bias2_ap,
              q2_ap, wst_ap, D, H, W, repeat=1):
    nc = tc.nc
    OD, OH, OW = D - 2, H - 2, W - 2
    NE = OD // 2                      # od pairs
    # h blocks: psum bank holds 512 fp32 -> max 11 rows of OW=46
    blocks = []
    r0 = 0
    while r0 < OH:
        R = min(11, OH - r0)
        blocks.append((r0, R))
        r0 += R

    const_pool = ctx.enter_context(tc.tile_pool(name="const", bufs=1))
    prep_psum = ctx.enter_context(
        tc.tile_pool(name="prep_psum", bufs=1, space="PSUM"))
    conv_psum = ctx.enter_context(
        tc.tile_pool(name="conv_psum", bufs=7, space="PSUM"))
    xtile_pool = ctx.enter_context(tc.tile_pool(name="xt", bufs=6))
    out_pool = ctx.enter_context(tc.tile_pool(name="outs", bufs=2))

    def body(_i=None):
        # ---------------- prep ----------------------------------------------
        stin = const_pool.tile([128, 4], F32, tag="stin")
        for c in range(4):
            nc.sync.dma_start(stin[:, c:c + 1], st4_ap[c])
        swt_t = const_pool.tile([128, 256], F32, tag="swt")
        for c in range(4):
            nc.sync.dma_start(swt_t[:, c * 64:(c + 1) * 64], swt_ap[c])
        stb2_t = const_pool.tile([128, 1], F32, tag="stb2")
        nc.sync.dma_start(stb2_t[:], stb2_ap[:])
        bias2_t = const_pool.tile([128, 1], F32, tag="bias2")
        nc.sync.dma_start(bias2_t[:], bias2_ap[:])
        q2_t = const_pool.tile([128, 128], F32, tag="q2")
        nc.sync.dma_start(q2_t[0:64, :], q2_ap[:])
        wst_t = const_pool.tile([128, NTAP, 128], F32, tag="wst")
        nc.sync.dma_start(wst_t[:], wst_ap[:])
        eps_t = const_pool.tile([128, 1], F32, tag="eps")
        nc.vector.memset(eps_t[:], EPS)

        # s = style @ style_w.T + style_b   (both partition halves)
        psum_s = prep_psum.tile([128, 1], F32, tag="prep")
        for half in (0, 64):
            for c in range(4):
                nc.tensor.matmul(
                    psum_s[half:half + 64, :],
                    lhsT=swt_t[:, c * 64:(c + 1) * 64],
                    rhs=stin[:, c:c + 1],
                    start=(c == 0), stop=(c == 3))
        s_col = const_pool.tile([128, 1], F32, tag="scol")
        nc.vector.tensor_add(s_col[:], psum_s[:], stb2_t[:])

        # modulated lhsT tiles (fp16), one DVE op for all 18
        wmod = const_pool.tile([128, NTAP, 128], F16, tag="wmod")
        nc.vector.tensor_scalar_mul(wmod[:], wst_t[:], s_col[:])

        # demod: var[m] = sum_ci s[ci]^2 q2[ci, m]  -> rsqrt
        s2 = const_pool.tile([128, 1], F32, tag="s2")
        nc.vector.tensor_mul(s2[0:64, :], s_col[0:64, :], s_col[0:64, :])
        psum_var = prep_psum.tile([128, 1], F32, tag="prep")
        nc.tensor.matmul(psum_var[:], lhsT=q2_t[0:64, :], rhs=s2[0:64, :],
                         start=True, stop=True)
        std_t = const_pool.tile([128, 1], F32, tag="std")
        nc.scalar.activation(std_t[:], psum_var[:],
                             mybir.ActivationFunctionType.Sqrt,
                             bias=eps_t[:])
        d_col = const_pool.tile([128, 1], F32, tag="dcol")
        nc.vector.reciprocal(d_col[:], std_t[:])

        # ---------------- conv ----------------------------------------------
        for e in range(NE):
            dt = xtile_pool.tile([128, H, W], F16, tag="dt", name="dt")
            nc.gpsimd.dma_start(dt[0:64, :, :], x_ap[:, 2 * e + 1, :, :])
            nc.gpsimd.dma_start(dt[64:128, :, :], x_ap[:, 2 * e + 2, :, :])
            rt = xtile_pool.tile([128, H, W], F16, tag="rt", name="rt")
            nc.gpsimd.dma_start(rt[0:64, :, :], x_ap[:, 2 * e, :, :])
            nc.gpsimd.dma_start(rt[64:128, :, :], x_ap[:, 2 * e + 3, :, :])

            psums = [
                conv_psum.tile([128, 512], F32, tag="cps", name="cps")
                [:, 0:R * OW].rearrange("p (r w) -> p r w", w=OW)
                for (_r0, R) in blocks]
            for t in range(NTAP):
                kh, kw = divmod(t % 9, 3)
                src = dt if t < 9 else rt
                lhsT = wmod[:, t, :]
                first = (t == 0)
                last = (t == NTAP - 1)
                for j, (r0, R) in enumerate(blocks):
                    nc.tensor.matmul(
                        psums[j],
                        lhsT=lhsT,
                        rhs=src[:, r0 + kh:r0 + kh + R, kw:kw + OW],
                        start=first, stop=last,
                        skip_group_check=True)

            ot = out_pool.tile([128, OH, OW], F32, tag="ot")
            for j, (r0, R) in enumerate(blocks):
                nc.vector.tensor_scalar(
                    out=ot[:, r0:r0 + R, :], in0=psums[j],
                    scalar1=d_col[:], scalar2=bias2_t[:],
                    op0=mybir.AluOpType.mult, op1=mybir.AluOpType.add)
            nc.sync.dma_start(y_ap[:, 2 * e, :, :], ot[0:64, :, :])
            nc.sync.dma_start(y_ap[:, 2 * e + 1, :, :], ot[64:128, :, :])

    if repeat == 1:
        body()
    else:
        with tc.For_i(0, repeat, 1) as i:
            body(i)


def build_bass(D=48, H=48, W=48, repeat=1, n_cores=N_CORES):
    from contextlib import ExitStack
    nc = bacc.Bacc("TRN2", target_bir_lowering=False, debug=False,
                   num_devices=n_cores)
    OD, OH, OW = D - 2, H - 2, W - 2
    x_ap = nc.dram_tensor("x", [CIN, D, H, W], F32, kind="ExternalInput").ap()
    st4_ap = nc.dram_tensor("st4", [4, 128, 1], F32, kind="ExternalInput").ap()
    swt_ap = nc.dram_tensor("swt", [4, 128, 64], F32, kind="ExternalInput").ap()
    stb2_ap = nc.dram_tensor("stb2", [128, 1], F32, kind="ExternalInput").ap()
    bias2_ap = nc.dram_tensor("bias2", [128, 1], F32,
                              kind="ExternalInput").ap()
    q2_ap = nc.dram_tensor("q2", [64, 128], F32, kind="ExternalInput").ap()
    wst_ap = nc.dram_tensor("wst", [128, NTAP, 128], F32,
                            kind="ExternalInput").ap()
    y_ap = nc.dram_tensor("y", [COUT, OD, OH, OW], F32,
                          kind="ExternalOutput").ap()
    with tile.TileContext(nc) as tc:
        with ExitStack() as ctx:
            conv_body(ctx, tc, y_ap, x_ap, st4_ap, swt_ap, stb2_ap, bias2_ap,
                      q2_ap, wst_ap, D, H, W, repeat=repeat)
    nc.compile()
    return nc


def make_in_maps(x, style, weight, bias, style_w, style_b):
    B = x.shape[0]
    w = np.asarray(weight, np.float32)
    # base lhsT tiles [18, 128, 128]: rows = (slice blk, ci), cols = (od blk, co)
    wt = np.transpose(w, (1, 0, 2, 3, 4))          # [ci, co, kd, kh, kw]
    Z = np.zeros((CIN, COUT), np.float32)
    tiles = []
    for kh in range(KK):
        for kw in range(KK):
            W0, W1, W2 = wt[:, :, 0, kh, kw], wt[:, :, 1, kh, kw], \
                wt[:, :, 2, kh, kw]
            tiles.append(np.block([[W1, W0], [W2, W1]]))
    for kh in range(KK):
        for kw in range(KK):
            W0, W2 = wt[:, :, 0, kh, kw], wt[:, :, 2, kh, kw]
            tiles.append(np.block([[W0, Z], [Z, W2]]))
    wst = np.ascontiguousarray(
        np.transpose(np.stack(tiles), (1, 0, 2)).astype(np.float32))

    q = (w ** 2).sum(axis=(2, 3, 4)).T             # [ci, co]
    q2 = np.ascontiguousarray(np.concatenate([q, q], axis=1)
                              .astype(np.float32))  # [64, 128]
    swt = np.ascontiguousarray(
        style_w.T.reshape(4, 128, 64).astype(np.float32))
    stb2 = np.ascontiguousarray(
        np.tile(style_b.reshape(64, 1), (2, 1)).astype(np.float32))
    bi2 = np.ascontiguousarray(
        np.tile(bias.reshape(64, 1), (2, 1)).astype(np.float32))
    return [{
        "x": np.ascontiguousarray(x[b].astype(np.float32)),
        "st4": np.ascontiguousarray(style[b].reshape(4, 128, 1)
                                    .astype(np.float32)),
        "swt": swt, "stb2": stb2, "bias2": bi2, "q2": q2, "wst": wst,
    } for b in range(B)]


_NC_CACHE = {}


def _get_nc(repeat=1):
    key = repeat
    if key not in _NC_CACHE:
        _NC_CACHE[key] = build_bass(48, 48, 48, repeat=repeat)
    return _NC_CACHE[key]


def kernel(x, style, weight, bias, style_w, style_b):
    assert x.shape == (8, CIN, 48, 48, 48), x.shape
    nc = _get_nc(1)
    in_maps = make_in_maps(x, style, weight, bias, style_w, style_b)
    res = run_bass_kernel_spmd(nc, in_maps, list(range(N_CORES)))
    y = np.stack([res.results[b]["y"] for b in range(len(in_maps))], axis=0)
    return y.astype(np.float32)


# revision 6
# speedup vs baseline: 1.1406x; 1.1406x over previous
"""Trainium2 Bass kernel for style-modulated 3D conv (DMSRStyleConv).

Math (per sample b):
  s[ci]   = style[b] @ style_w.T + style_b                    # [Cin]
  w_mod   = weight * s[None, :, None,None,None]               # [Cout,Cin,3,3,3]
  var[co] = sum_{ci,taps} w_mod^2 = sum_ci s[ci]^2 * q[ci,co]
  y[b]    = conv3d_valid(x[b], w_mod) * rsqrt(var+eps)[co] + bias[co]

Strategy: data-parallel over batch across 8 cores (1 sample each).

PE formulation: all matmuls are full-array K=128 x M=128.  The
depth (d) axis is paired in the contraction dim and the output depth
(od) axis is paired in the PSUM partition dim:

  rhs "D" tile = [x[2e+1] ; x[2e+2]]  (128 partitions = 2 slices x 64 cin)
  rhs "R" tile = [x[2e]   ; x[2e+3]]
  psum tile    = [y[2e]   ; y[2e+1]]  (128 partitions = 2 od x 64 cout)

Per od-pair e and per (kh,kw) there are two passes:
  dense (rhs=D): lhsT = [[W1  W0],[W2  W1]]   (all 4 blocks useful)
  rem   (rhs=R): lhsT = [[W0   0],[ 0  W2]]
giving 18 passes x 2116 spatial columns per od-pair = 876k psum columns
total (vs 2.63M for a 64x64-quadrant formulation).

Weights are static across the sample's conv: 18 base lhsT tiles are
precomputed on the host, modulated once per kernel by s[ci] (one DVE op)
into fp16.  x slices are cast fp32->fp16 during the DMA itself (SWDGE).
Demod scale and bias are folded into the PSUM->SBUF eviction.
"""

import numpy as np

import concourse.bass as bass
import concourse.tile as tile
from concourse import bacc, mybir
from concourse.bass_utils import run_bass_kernel_spmd

F32 = mybir.dt.float32
F16 = mybir.dt.float16
EPS = 1e-8
N_CORES = 8
CIN = 64
COUT = 64
KK = 3
NTAP = 18          # 9 dense + 9 remainder lhsT tiles


def conv_body(ctx, tc, y_ap, x_ap, st4_ap, swt_ap, stb2_ap, bias2_ap,
              q2_ap, wst_ap, D, H, W, repeat=1):
    nc = tc.nc
    OD, OH, OW = D - 2, H - 2, W - 2
    NE = OD // 2                      # od pairs
    # h blocks: psum bank holds 512 fp32 -> max 11 rows of OW=46
    blocks = []
    r0 = 0
    while r0 < OH:
        R = min(11, OH - r0)
        blocks.append((r0, R))
        r0 += R

    const_pool = ctx.enter_context(tc.tile_pool(name="const", bufs=1))
    prep_psum = ctx.enter_context(
        tc.tile_pool(name="prep_psum", bufs=1, space="PSUM"))
    conv_psum = ctx.enter_context(
        tc.tile_pool(name="conv_psum", bufs=7, space="PSUM"))
    stg_pool = ctx.enter_context(tc.tile_pool(name="stg", bufs=4))
    xtile_pool = ctx.enter_context(tc.tile_pool(name="xt", bufs=6))
    out_pool = ctx.enter_context(tc.tile_pool(name="outs", bufs=2))

    def body(_i=None):
        # ---------------- prep ----------------------------------------------
        stin = const_pool.tile([128, 4], F32, tag="stin")
        for c in range(4):
            nc.sync.dma_start(stin[:, c:c + 1], st4_ap[c])
        swt_t = const_pool.tile([128, 256], F32, tag="swt")
        for c in range(4):
            nc.sync.dma_start(swt_t[:, c * 64:(c + 1) * 64], swt_ap[c])
        stb2_t = const_pool.tile([128, 1], F32, tag="stb2")
        nc.sync.dma_start(stb2_t[:], stb2_ap[:])
        bias2_t = const_pool.tile([128, 1], F32, tag="bias2")
        nc.sync.dma_start(bias2_t[:], bias2_ap[:])
        q2_t = const_pool.tile([128, 128], F32, tag="q2")
        nc.sync.dma_start(q2_t[0:64, :], q2_ap[:])
        wst_t = const_pool.tile([128, NTAP, 128], F32, tag="wst")
        nc.sync.dma_start(wst_t[:], wst_ap[:])
        eps_t = const_pool.tile([128, 1], F32, tag="eps")
        nc.vector.memset(eps_t[:], EPS)

        # s = style @ style_w.T + style_b   (both partition halves)
        psum_s = prep_psum.tile([128, 1], F32, tag="prep")
        for half in (0, 64):
            for c in range(4):
                nc.tensor.matmul(
                    psum_s[half:half + 64, :],
                    lhsT=swt_t[:, c * 64:(c + 1) * 64],
                    rhs=stin[:, c:c + 1],
                    start=(c == 0), stop=(c == 3))
        s_col = const_pool.tile([128, 1], F32, tag="scol")
        nc.vector.tensor_add(s_col[:], psum_s[:], stb2_t[:])

        # modulated lhsT tiles (fp16), one DVE op for all 18
        wmod = const_pool.tile([128, NTAP, 128], F16, tag="wmod")
        nc.vector.tensor_scalar_mul(wmod[:], wst_t[:], s_col[:])

        # demod: var[m] = sum_ci s[ci]^2 q2[ci, m]  -> rsqrt
        s2 = const_pool.tile([128, 1], F32, tag="s2")
        nc.vector.tensor_mul(s2[0:64, :], s_col[0:64, :], s_col[0:64, :])
        psum_var = prep_psum.tile([128, 1], F32, tag="prep")
        nc.tensor.matmul(psum_var[:], lhsT=q2_t[0:64, :], rhs=s2[0:64, :],
                         start=True, stop=True)
        std_t = const_pool.tile([128, 1], F32, tag="std")
        nc.scalar.activation(std_t[:], psum_var[:],
                             mybir.ActivationFunctionType.Sqrt,
                             bias=eps_t[:])
        d_col = const_pool.tile([128, 1], F32, tag="dcol")
        nc.vector.reciprocal(d_col[:], std_t[:])

        # ---------------- conv ----------------------------------------------
        def load_pair(sa, sb, tag):
            stg = stg_pool.tile([128, H, W], F32, tag="stg_" + tag,
                                name="stg_" + tag)
            nc.sync.dma_start(stg[0:64, :, :], x_ap[:, sa, :, :])
            nc.sync.dma_start(stg[64:128, :, :], x_ap[:, sb, :, :])
            t = xtile_pool.tile([128, H, W], F16, tag=tag, name=tag)
            nc.vector.tensor_copy(t[:, :, :], stg[:, :, :])
            return t

        for e in range(NE):
            dt = load_pair(2 * e + 1, 2 * e + 2, "dt")
            rt = load_pair(2 * e, 2 * e + 3, "rt")

            psums = [
                conv_psum.tile([128, 512], F32, tag="cps", name="cps")
                [:, 0:R * OW].rearrange("p (r w) -> p r w", w=OW)
                for (_r0, R) in blocks]
            for t in range(NTAP):
                kh, kw = divmod(t % 9, 3)
                src = dt if t < 9 else rt
                lhsT = wmod[:, t, :]
                first = (t == 0)
                last = (t == NTAP - 1)
                for j, (r0, R) in enumerate(blocks):
                    nc.tensor.matmul(
                        psums[j],
                        lhsT=lhsT,
                        rhs=src[:, r0 + kh:r0 + kh + R, kw:kw + OW],
                        start=first, stop=last,
                        skip_group_check=True)

            ot = out_pool.tile([128, OH, OW], F32, tag="ot")
            for j, (r0, R) in enumerate(blocks):
                nc.vector.tensor_scalar(
                    out=ot[:, r0:r0 + R, :], in0=psums[j],
                    scalar1=d_col[:], scalar2=bias2_t[:],
                    op0=mybir.AluOpType.mult, op1=mybir.AluOpType.add)
            nc.scalar.dma_start(y_ap[:, 2 * e, :, :], ot[0:64, :, :])
            nc.scalar.dma_start(y_ap[:, 2 * e + 1, :, :], ot[64:128, :, :])

    if repeat == 1:
        body()
    else:
        with tc.For_i(0, repeat, 1) as i:
            body(i)


def build_bass(D=48, H=48, W=48, repeat=1, n_cores=N_CORES):
    from contextlib import ExitStack
    nc = bacc.Bacc("TRN2", target_bir_lowering=False, debug=False,
                   num_devices=n_cores)
    OD, OH, OW = D - 2, H - 2, W - 2
    x_ap = nc.dram_tensor("x", [CIN, D, H, W], F32, kind="ExternalInput").ap()
    st4_ap = nc.dram_tensor("st4", [4, 128, 1], F32, kind="ExternalInput").ap()
    swt_ap = nc.dram_tensor("swt", [4, 128, 64], F32, kind="ExternalInput").ap()
    stb2_ap = nc.dram_tensor("stb2", [128, 1], F32, kind="ExternalInput").ap()
    bias2_ap = nc.dram_tensor("bias2", [128, 1], F32,
                              kind="ExternalInput").ap()
    q2_ap = nc.dram_tensor("q2", [64, 128], F32, kind="ExternalInput").ap()
    wst_ap = nc.dram_tensor("wst", [128, NTAP, 128], F32,
                            kind="ExternalInput").ap()
    y_ap = nc.dram_tensor("y", [COUT, OD, OH, OW], F32,
                          kind="ExternalOutput").ap()
    with tile.TileContext(nc) as tc:
        with ExitStack() as ctx:
            conv_body(ctx, tc, y_ap, x_ap, st4_ap, swt_ap, stb2_ap, bias2_ap,
                      q2_ap, wst_ap, D, H, W, repeat=repeat)
    nc.compile()
    return nc


def make_in_maps(x, style, weight, bias, style_w, style_b):
    B = x.shape[0]
    w = np.asarray(weight, np.float32)
    # base lhsT tiles [18, 128, 128]: rows = (slice blk, ci), cols = (od blk, co)
    wt = np.transpose(w, (1, 0, 2, 3, 4))          # [ci, co, kd, kh, kw]
    Z = np.zeros((CIN, COUT), np.float32)
    tiles = []
    for kh in range(KK):
        for kw in range(KK):
            W0, W1, W2 = wt[:, :, 0, kh, kw], wt[:, :, 1, kh, kw], \
                wt[:, :, 2, kh, kw]
            tiles.append(np.block([[W1, W0], [W2, W1]]))
    for kh in range(KK):
        for kw in range(KK):
            W0, W2 = wt[:, :, 0, kh, kw], wt[:, :, 2, kh, kw]
            tiles.append(np.block([[W0, Z], [Z, W2]]))
    wst = np.ascontiguousarray(
        np.transpose(np.stack(tiles), (1, 0, 2)).astype(np.float32))

    q = (w ** 2).sum(axis=(2, 3, 4)).T             # [ci, co]
    q2 = np.ascontiguousarray(np.concatenate([q, q], axis=1)
                              .astype(np.float32))  # [64, 128]
    swt = np.ascontiguousarray(
        style_w.T.reshape(4, 128, 64).astype(np.float32))
    stb2 = np.ascontiguousarray(
        np.tile(style_b.reshape(64, 1), (2, 1)).astype(np.float32))
    bi2 = np.ascontiguousarray(
        np.tile(bias.reshape(64, 1), (2, 1)).astype(np.float32))
    return [{
        "x": np.ascontiguousarray(x[b].astype(np.float32)),
        "st4": np.ascontiguousarray(style[b].reshape(4, 128, 1)
                                    .astype(np.float32)),
        "swt": swt, "stb2": stb2, "bias2": bi2, "q2": q2, "wst": wst,
    } for b in range(B)]


_NC_CACHE = {}


def _get_nc(repeat=1):
    key = repeat
    if key not in _NC_CACHE:
        _NC_CACHE[key] = build_bass(48, 48, 48, repeat=repeat)
    return _NC_CACHE[key]


def kernel(x, style, weight, bias, style_w, style_b):
    assert x.shape == (8, CIN, 48, 48, 48), x.shape
    nc = _get_nc(1)
    in_maps = make_in_maps(x, style, weight, bias, style_w, style_b)
    res = run_bass_kernel_spmd(nc, in_maps, list(range(N_CORES)))
    y = np.stack([res.results[b]["y"] for b in range(len(in_maps))], axis=0)
    return y.astype(np.float32)


# revision 12
# speedup vs baseline: 2.2385x; 1.9625x over previous
"""Trainium2 Bass kernel for style-modulated 3D conv (DMSRStyleConv).

Math (per sample b):
  s[ci]   = style[b] @ style_w.T + style_b                    # [Cin]
  w_mod   = weight * s[None, :, None,None,None]               # [Cout,Cin,3,3,3]
  var[co] = sum_{ci,taps} w_mod^2 = sum_ci s[ci]^2 * q[ci,co]
  y[b]    = conv3d_valid(x[b], w_mod) * rsqrt(var+eps)[co] + bias[co]

Strategy: data-parallel over batch across 8 cores (1 sample each).

PE formulation: all matmuls are full-array K=128 x M=128.  The
depth (d) axis is paired in the contraction dim and the output depth
(od) axis is paired in the PSUM partition dim:

  rhs "D" tile = [x[2e+1] ; x[2e+2]]  (128 partitions = 2 slices x 64 cin)
  rhs "R" tile = [x[2e]   ; x[2e+3]]
  psum tile    = [y[2e]   ; y[2e+1]]  (128 partitions = 2 od x 64 cout)

Per od-pair e and per (kh,kw) there are two passes:
  dense (rhs=D): lhsT = [[W1  W0],[W2  W1]]   (all 4 blocks useful)
  rem   (rhs=R): lhsT = [[W0   0],[ 0  W2]]
giving 18 passes x 2116 spatial columns per od-pair = 876k psum columns
total (vs 2.63M for a 64x64-quadrant formulation).

Weights are static across the sample's conv: 18 base lhsT tiles are
precomputed on the host, modulated once per kernel by s[ci] (one DVE op)
into fp16.  x is shipped as fp16 from the host, so slice loads are plain
HWDGE DMAs straight into the pair tiles.  Demod scale and bias are folded
into the PSUM->SBUF eviction on the scalar engine.
"""

import numpy as np

import concourse.bass as bass
import concourse.tile as tile
from concourse import bacc, mybir
from concourse.bass_utils import run_bass_kernel_spmd

F32 = mybir.dt.float32
F16 = mybir.dt.float16
EPS = 1e-8
N_CORES = 8
CIN = 64
COUT = 64
KK = 3
NTAP = 18          # 9 dense + 9 remainder lhsT tiles


def conv_body(ctx, tc, y_ap, x_ap, st4_ap, swt_ap, stb2_ap, bias2_ap,
              q2_ap, wst_ap, D, H, W, repeat=1):
    nc = tc.nc
    OD, OH, OW = D - 2, H - 2, W - 2
    NE = OD // 2                      # od pairs
    # h blocks: psum bank holds 512 fp32 -> max 11 rows of OW=46
    blocks = []
    r0 = 0
    while r0 < OH:
        R = min(11, OH - r0)
        blocks.append((r0, R))
        r0 += R

    const_pool = ctx.enter_context(tc.tile_pool(name="const", bufs=1))
    prep_psum = ctx.enter_context(
        tc.tile_pool(name="prep_psum", bufs=1, space="PSUM"))
    conv_psum = ctx.enter_context(
        tc.tile_pool(name="conv_psum", bufs=7, space="PSUM"))
    xtile_pool = ctx.enter_context(tc.tile_pool(name="xt", bufs=6))
    out_pool = ctx.enter_context(tc.tile_pool(name="outs", bufs=2))

    def body(_i=None):
        # ---------------- prep ----------------------------------------------
        stin = const_pool.tile([128, 4], F32, tag="stin")
        for c in range(4):
            nc.sync.dma_start(stin[:, c:c + 1], st4_ap[c])
        swt_t = const_pool.tile([128, 256], F32, tag="swt")
        for c in range(4):
            nc.sync.dma_start(swt_t[:, c * 64:(c + 1) * 64], swt_ap[c])
        stb2_t = const_pool.tile([128, 1], F32, tag="stb2")
        nc.sync.dma_start(stb2_t[:], stb2_ap[:])
        bias2_t = const_pool.tile([128, 1], F32, tag="bias2")
        nc.sync.dma_start(bias2_t[:], bias2_ap[:])
        q2_t = const_pool.tile([128, 128], F32, tag="q2")
        nc.sync.dma_start(q2_t[0:64, :], q2_ap[:])
        wst_t = const_pool.tile([128, NTAP, 128], F32, tag="wst")
        nc.sync.dma_start(wst_t[:], wst_ap[:])
        eps_t = const_pool.tile([128, 1], F32, tag="eps")
        nc.vector.memset(eps_t[:], EPS)

        # s = style @ style_w.T + style_b   (both partition halves)
        psum_s = prep_psum.tile([128, 1], F32, tag="prep")
        for half in (0, 64):
            for c in range(4):
                nc.tensor.matmul(
                    psum_s[half:half + 64, :],
                    lhsT=swt_t[:, c * 64:(c + 1) * 64],
                    rhs=stin[:, c:c + 1],
                    start=(c == 0), stop=(c == 3))
        s_col = const_pool.tile([128, 1], F32, tag="scol")
        nc.vector.tensor_add(s_col[:], psum_s[:], stb2_t[:])

        # modulated lhsT tiles (fp16), one DVE op for all 18
        wmod = const_pool.tile([128, NTAP, 128], F16, tag="wmod")
        nc.vector.tensor_scalar_mul(wmod[:], wst_t[:], s_col[:])

        # demod: var[m] = sum_ci s[ci]^2 q2[ci, m]  -> rsqrt
        s2 = const_pool.tile([128, 1], F32, tag="s2")
        nc.vector.tensor_mul(s2[0:64, :], s_col[0:64, :], s_col[0:64, :])
        psum_var = prep_psum.tile([128, 1], F32, tag="prep")
        nc.tensor.matmul(psum_var[:], lhsT=q2_t[0:64, :], rhs=s2[0:64, :],
                         start=True, stop=True)
        std_t = const_pool.tile([128, 1], F32, tag="std")
        nc.scalar.activation(std_t[:], psum_var[:],
                             mybir.ActivationFunctionType.Sqrt,
                             bias=eps_t[:])
        d_col = const_pool.tile([128, 1], F32, tag="dcol")
        nc.vector.reciprocal(d_col[:], std_t[:])

        # ---------------- conv ----------------------------------------------
        def load_pair(sa, sb, tag):
            t = xtile_pool.tile([128, H, W], F16, tag=tag, name=tag)
            nc.sync.dma_start(t[0:64, :, :], x_ap[:, sa, :, :])
            nc.sync.dma_start(t[64:128, :, :], x_ap[:, sb, :, :])
            return t

        for e in range(NE):
            dt = load_pair(2 * e + 1, 2 * e + 2, "dt")
            rt = load_pair(2 * e, 2 * e + 3, "rt")

            psums = [
                conv_psum.tile([128, 512], F32, tag="cps", name="cps")
                [:, 0:R * OW].rearrange("p (r w) -> p r w", w=OW)
                for (_r0, R) in blocks]
            for t in range(NTAP):
                kh, kw = divmod(t % 9, 3)
                src = dt if t < 9 else rt
                lhsT = wmod[:, t, :]
                first = (t == 0)
                last = (t == NTAP - 1)
                for j, (r0, R) in enumerate(blocks):
                    nc.tensor.matmul(
                        psums[j],
                        lhsT=lhsT,
                        rhs=src[:, r0 + kh:r0 + kh + R, kw:kw + OW],
                        start=first, stop=last,
                        skip_group_check=True)

            ot = out_pool.tile([128, OH, OW], F32, tag="ot")
            for j, (r0, R) in enumerate(blocks):
                nc.scalar.activation(
                    ot[:, r0:r0 + R, :], psums[j],
                    mybir.ActivationFunctionType.Identity,
                    scale=d_col[:], bias=bias2_t[:])
            nc.scalar.dma_start(y_ap[:, 2 * e, :, :], ot[0:64, :, :])
            nc.scalar.dma_start(y_ap[:, 2 * e + 1, :, :], ot[64:128, :, :])

    if repeat == 1:
        body()
    else:
        with tc.For_i(0, repeat, 1) as i:
            body(i)


def build_bass(D=48, H=48, W=48, repeat=1, n_cores=N_CORES):
    from contextlib import ExitStack
    nc = bacc.Bacc("TRN2", target_bir_lowering=False, debug=False,
                   num_devices=n_cores)
    OD, OH, OW = D - 2, H - 2, W - 2
    x_ap = nc.dram_tensor("x", [CIN, D, H, W], F16, kind="ExternalInput").ap()
    st4_ap = nc.dram_tensor("st4", [4, 128, 1], F32, kind="ExternalInput").ap()
    swt_ap = nc.dram_tensor("swt", [4, 128, 64], F32, kind="ExternalInput").ap()
    stb2_ap = nc.dram_tensor("stb2", [128, 1], F32, kind="ExternalInput").ap()
    bias2_ap = nc.dram_tensor("bias2", [128, 1], F32,
                              kind="ExternalInput").ap()
    q2_ap = nc.dram_tensor("q2", [64, 128], F32, kind="ExternalInput").ap()
    wst_ap = nc.dram_tensor("wst", [128, NTAP, 128], F32,
                            kind="ExternalInput").ap()
    y_ap = nc.dram_tensor("y", [COUT, OD, OH, OW], F32,
                          kind="ExternalOutput").ap()
    with tile.TileContext(nc) as tc:
        with ExitStack() as ctx:
            conv_body(ctx, tc, y_ap, x_ap, st4_ap, swt_ap, stb2_ap, bias2_ap,
                      q2_ap, wst_ap, D, H, W, repeat=repeat)
    nc.compile()
    return nc


def make_in_maps(x, style, weight, bias, style_w, style_b):
    B = x.shape[0]
    w = np.asarray(weight, np.float32)
    # base lhsT tiles [18, 128, 128]: rows = (slice blk, ci), cols = (od blk, co)
    wt = np.transpose(w, (1, 0, 2, 3, 4))          # [ci, co, kd, kh, kw]
    Z = np.zeros((CIN, COUT), np.float32)
    tiles = []
    for kh in range(KK):
        for kw in range(KK):
            W0, W1, W2 = wt[:, :, 0, kh, kw], wt[:, :, 1, kh, kw], \
                wt[:, :, 2, kh, kw]
            tiles.append(np.block([[W1, W0], [W2, W1]]))
    for kh in range(KK):
        for kw in range(KK):
            W0, W2 = wt[:, :, 0, kh, kw], wt[:, :, 2, kh, kw]
            tiles.append(np.block([[W0, Z], [Z, W2]]))
    wst = np.ascontiguousarray(
        np.transpose(np.stack(tiles), (1, 0, 2)).astype(np.float32))

    q = (w ** 2).sum(axis=(2, 3, 4)).T             # [ci, co]
    q2 = np.ascontiguousarray(np.concatenate([q, q], axis=1)
                              .astype(np.float32))  # [64, 128]
    swt = np.ascontiguousarray(
        style_w.T.reshape(4, 128, 64).astype(np.float32))
    stb2 = np.ascontiguousarray(
        np.tile(style_b.reshape(64, 1), (2, 1)).astype(np.float32))
    bi2 = np.ascontiguousarray(
        np.tile(bias.reshape(64, 1), (2, 1)).astype(np.float32))
    return [{
        "x": np.ascontiguousarray(x[b].astype(np.float16)),
        "st4": np.ascontiguousarray(style[b].reshape(4, 128, 1)
                                    .astype(np.float32)),
        "swt": swt, "stb2": stb2, "bias2": bi2, "q2": q2, "wst": wst,
    } for b in range(B)]


_NC_CACHE = {}


def _get_nc(repeat=1):
    key = repeat
    if key not in _NC_CACHE:
        _NC_CACHE[key] = build_bass(48, 48, 48, repeat=repeat)
    return _NC_CACHE[key]


def kernel(x, style, weight, bias, style_w, style_b):
    assert x.shape == (8, CIN, 48, 48, 48), x.shape
    nc = _get_nc(1)
    in_maps = make_in_maps(x, style, weight, bias, style_w, style_b)
    res = run_bass_kernel_spmd(nc, in_maps, list(range(N_CORES)))
    y = np.stack([res.results[b]["y"] for b in range(len(in_maps))], axis=0)
    return y.astype(np.float32)


# revision 14
# speedup vs baseline: 2.4775x; 1.1068x over previous
"""Trainium2 Bass kernel for style-modulated 3D conv (DMSRStyleConv).

Math (per sample b):
  s[ci]   = style[b] @ style_w.T + style_b                    # [Cin]
  w_mod   = weight * s[None, :, None,None,None]               # [Cout,Cin,3,3,3]
  var[co] = sum_{ci,taps} w_mod^2 = sum_ci s[ci]^2 * q[ci,co]
  y[b]    = conv3d_valid(x[b], w_mod) * rsqrt(var+eps)[co] + bias[co]

Strategy: data-parallel over batch across 8 cores (1 sample each).

PE formulation: all matmuls are full-array K=128 x M=128.  The
depth (d) axis is paired in the contraction dim and the output depth
(od) axis is paired in the PSUM partition dim:

  rhs "D" tile = [x[2e+1] ; x[2e+2]]  (128 partitions = 2 slices x 64 cin)
  rhs "R" tile = [x[2e]   ; x[2e+3]]
  psum tile    = [y[2e]   ; y[2e+1]]  (128 partitions = 2 od x 64 cout)

Per od-pair e and per (kh,kw) there are two passes:
  dense (rhs=D): lhsT = [[W1  W0],[W2  W1]]   (all 4 blocks useful)
  rem   (rhs=R): lhsT = [[W0   0],[ 0  W2]]
giving 18 passes x 2116 spatial columns per od-pair = 876k psum columns
total (vs 2.63M for a 64x64-quadrant formulation).

Weights are static across the sample's conv: 18 base lhsT tiles are
precomputed on the host, modulated once per kernel by s[ci] (one DVE op)
into fp16.  x is shipped as fp16 from the host, so slice loads are plain
HWDGE DMAs straight into the pair tiles.  Demod scale and bias are folded
into the PSUM->SBUF eviction on the scalar engine.
"""

import numpy as np

import concourse.bass as bass
import concourse.tile as tile
from concourse import bacc, mybir
from concourse.bass_utils import run_bass_kernel_spmd

F32 = mybir.dt.float32
F16 = mybir.dt.float16
EPS = 1e-8
N_CORES = 8
CIN = 64
COUT = 64
KK = 3
NTAP = 18          # 9 dense + 9 remainder lhsT tiles


def conv_body(ctx, tc, y_ap, x_ap, st4_ap, swt_ap, stb2_ap, bias2_ap,
              q2_ap, wst_ap, D, H, W, repeat=1):
    nc = tc.nc
    OD, OH, OW = D - 2, H - 2, W - 2
    NE = OD // 2                      # od pairs
    # h blocks: psum bank holds 512 fp32 -> max 11 rows of OW=46
    blocks = []
    r0 = 0
    while r0 < OH:
        R = min(11, OH - r0)
        blocks.append((r0, R))
        r0 += R

    const_pool = ctx.enter_context(tc.tile_pool(name="const", bufs=1))
    prep_psum = ctx.enter_context(
        tc.tile_pool(name="prep_psum", bufs=1, space="PSUM"))
    conv_psum = ctx.enter_context(
        tc.tile_pool(name="conv_psum", bufs=7, space="PSUM"))
    xtile_pool = ctx.enter_context(tc.tile_pool(name="xt", bufs=6))
    out_pool = ctx.enter_context(tc.tile_pool(name="outs", bufs=2))

    def body(_i=None):
        # ---------------- prep ----------------------------------------------
        stin = const_pool.tile([128, 4], F32, tag="stin")
        for c in range(4):
            nc.sync.dma_start(stin[:, c:c + 1], st4_ap[c])
        swt_t = const_pool.tile([128, 256], F32, tag="swt")
        for c in range(4):
            nc.sync.dma_start(swt_t[:, c * 64:(c + 1) * 64], swt_ap[c])
        stb2_t = const_pool.tile([128, 1], F32, tag="stb2")
        nc.sync.dma_start(stb2_t[:], stb2_ap[:])
        bias2_t = const_pool.tile([128, 1], F32, tag="bias2")
        nc.sync.dma_start(bias2_t[:], bias2_ap[:])
        q2_t = const_pool.tile([128, 128], F32, tag="q2")
        nc.sync.dma_start(q2_t[0:64, :], q2_ap[:])
        wst_t = const_pool.tile([128, NTAP, 128], F32, tag="wst")
        nc.sync.dma_start(wst_t[:], wst_ap[:])
        eps_t = const_pool.tile([128, 1], F32, tag="eps")
        nc.vector.memset(eps_t[:], EPS)

        # s = style @ style_w.T + style_b   (both partition halves)
        psum_s = prep_psum.tile([128, 1], F32, tag="prep")
        for half in (0, 64):
            for c in range(4):
                nc.tensor.matmul(
                    psum_s[half:half + 64, :],
                    lhsT=swt_t[:, c * 64:(c + 1) * 64],
                    rhs=stin[:, c:c + 1],
                    start=(c == 0), stop=(c == 3))
        s_col = const_pool.tile([128, 1], F32, tag="scol")
        nc.vector.tensor_add(s_col[:], psum_s[:], stb2_t[:])

        # modulated lhsT tiles (fp16), one DVE op for all 18
        wmod = const_pool.tile([128, NTAP, 128], F16, tag="wmod")
        nc.vector.tensor_scalar_mul(wmod[:], wst_t[:], s_col[:])

        # demod: var[m] = sum_ci s[ci]^2 q2[ci, m]  -> rsqrt
        s2 = const_pool.tile([128, 1], F32, tag="s2")
        nc.vector.tensor_mul(s2[0:64, :], s_col[0:64, :], s_col[0:64, :])
        psum_var = prep_psum.tile([128, 1], F32, tag="prep")
        nc.tensor.matmul(psum_var[:], lhsT=q2_t[0:64, :], rhs=s2[0:64, :],
                         start=True, stop=True)
        std_t = const_pool.tile([128, 1], F32, tag="std")
        nc.scalar.activation(std_t[:], psum_var[:],
                             mybir.ActivationFunctionType.Sqrt,
                             bias=eps_t[:])
        d_col = const_pool.tile([128, 1], F32, tag="dcol")
        nc.vector.reciprocal(d_col[:], std_t[:])

        # ---------------- conv ----------------------------------------------
        def load_pair(sa, sb, tag):
            t = xtile_pool.tile([128, H, W], F16, tag=tag, name=tag)
            nc.sync.dma_start(t[0:64, :, :], x_ap[:, sa, :, :])
            nc.sync.dma_start(t[64:128, :, :], x_ap[:, sb, :, :])
            return t

        for e in range(NE):
            dt = load_pair(2 * e + 1, 2 * e + 2, "dt")
            rt = load_pair(2 * e, 2 * e + 3, "rt")

            psums = [
                conv_psum.tile([128, 512], F32, tag="cps", name="cps")
                [:, 0:R * OW].rearrange("p (r w) -> p r w", w=OW)
                for (_r0, R) in blocks]
            for t in range(NTAP):
                kh, kw = divmod(t % 9, 3)
                src = dt if t < 9 else rt
                lhsT = wmod[:, t, :]
                first = (t == 0)
                last = (t == NTAP - 1)
                for j, (r0, R) in enumerate(blocks):
                    nc.tensor.matmul(
                        psums[j],
                        lhsT=lhsT,
                        rhs=src[:, r0 + kh:r0 + kh + R, kw:kw + OW],
                        start=first, stop=last,
                        skip_group_check=True)

            ot = out_pool.tile([128, OH, OW], F32, tag="ot")
            for j, (r0, R) in enumerate(blocks):
                nc.scalar.activation(
                    ot[:, r0:r0 + R, :], psums[j],
                    mybir.ActivationFunctionType.Identity,
                    scale=d_col[:], bias=bias2_t[:])
            nc.scalar.dma_start(y_ap[:, 2 * e, :, :], ot[0:64, :, :])
            nc.scalar.dma_start(y_ap[:, 2 * e + 1, :, :], ot[64:128, :, :])

    if repeat == 1:
        body()
    else:
        with tc.For_i(0, repeat, 1) as i:
            body(i)


def build_bass(D=48, H=48, W=48, repeat=1, n_cores=N_CORES):
    from contextlib import ExitStack
    nc = bacc.Bacc("TRN2", target_bir_lowering=False, debug=False,
                   num_devices=n_cores)
    OD, OH, OW = D - 2, H - 2, W - 2
    x_ap = nc.dram_tensor("x", [CIN, D, H, W], F16, kind="ExternalInput").ap()
    st4_ap = nc.dram_tensor("st4", [4, 128, 1], F32, kind="ExternalInput").ap()
    swt_ap = nc.dram_tensor("swt", [4, 128, 64], F32, kind="ExternalInput").ap()
    stb2_ap = nc.dram_tensor("stb2", [128, 1], F32, kind="ExternalInput").ap()
    bias2_ap = nc.dram_tensor("bias2", [128, 1], F32,
                              kind="ExternalInput").ap()
    q2_ap = nc.dram_tensor("q2", [64, 128], F32, kind="ExternalInput").ap()
    wst_ap = nc.dram_tensor("wst", [128, NTAP, 128], F32,
                            kind="ExternalInput").ap()
    y_ap = nc.dram_tensor("y", [COUT, OD, OH, OW], F32,
                          kind="ExternalOutput").ap()
    with tile.TileContext(nc) as tc:
        with ExitStack() as ctx:
            conv_body(ctx, tc, y_ap, x_ap, st4_ap, swt_ap, stb2_ap, bias2_ap,
                      q2_ap, wst_ap, D, H, W, repeat=repeat)
    nc.compile()
    return nc


def make_in_maps(x, style, weight, bias, style_w, style_b):
    B = x.shape[0]
    w = np.asarray(weight, np.float32)
    # base lhsT tiles [18, 128, 128]: rows = (slice blk, ci), cols = (od blk, co)
    wt = np.transpose(w, (1, 0, 2, 3, 4))          # [ci, co, kd, kh, kw]
    Z = np.zeros((CIN, COUT), np.float32)
    tiles = []
    for kh in range(KK):
        for kw in range(KK):
            W0, W1, W2 = wt[:, :, 0, kh, kw], wt[:, :, 1, kh, kw], \
                wt[:, :, 2, kh, kw]
            tiles.append(np.block([[W1, W0], [W2, W1]]))
    for kh in range(KK):
        for kw in range(KK):
            W0, W2 = wt[:, :, 0, kh, kw], wt[:, :, 2, kh, kw]
            tiles.append(np.block([[W0, Z], [Z, W2]]))
    wst = np.ascontiguousarray(
        np.transpose(np.stack(tiles), (1, 0, 2)).astype(np.float32))

    q = (w ** 2).sum(axis=(2, 3, 4)).T             # [ci, co]
    q2 = np.ascontiguousarray(np.concatenate([q, q], axis=1)
                              .astype(np.float32))  # [64, 128]
    swt = np.ascontiguousarray(
        style_w.T.reshape(4, 128, 64).astype(np.float32))
    stb2 = np.ascontiguousarray(
        np.tile(style_b.reshape(64, 1), (2, 1)).astype(np.float32))
    bi2 = np.ascontiguousarray(
        np.tile(bias.reshape(64, 1), (2, 1)).astype(np.float32))
    return [{
        "x": np.ascontiguousarray(x[b].astype(np.float16)),
        "st4": np.ascontiguousarray(style[b].reshape(4, 128, 1)
                                    .astype(np.float32)),
        "swt": swt, "stb2": stb2, "bias2": bi2, "q2": q2, "wst": wst,
    } for b in range(B)]


_NC_CACHE = {}


def _get_nc(repeat=1):
    key = repeat
    if key not in _NC_CACHE:
        _NC_CACHE[key] = build_bass(48, 48, 48, repeat=repeat)
    return _NC_CACHE[key]


def kernel(x, style, weight, bias, style_w, style_b):
    assert x.shape == (8, CIN, 48, 48, 48), x.shape
    nc = _get_nc(1)
    in_maps = make_in_maps(x, style, weight, bias, style_w, style_b)
    res = run_bass_kernel_spmd(nc, in_maps, list(range(N_CORES)))
    y = np.stack([res.results[b]["y"] for b in range(len(in_maps))], axis=0)
    return y.astype(np.float32)
